# revision 1
# baseline (speedup 1.0000x reference)
# Trainium2 Bass kernels for nn_DecoderLayer (T5-style decoder layer).
# Two SPMD launches over 8 cores:
#   A: head-parallel attention (2 heads/core) -> per-head attn outputs O^T
#   B: token-parallel output-proj + FFN (512 tokens/core)
# Activations kept feature-major (features on partitions).
import sys
sys.path.insert(0, '/opt/trn_rl_repo')
import math
import numpy as np
import bass_rust
import concourse.bass as bass
import concourse.mybir as mybir
import concourse.tile as tile
from concourse import bacc

F32 = mybir.dt.float32
F32R = mybir.dt.float32r
AF = mybir.ActivationFunctionType

E = 1024
H = 16
D = 64
HID = 4096
B = 2
S = 2048
NB = 32
MAXD = 128
LN_EPS = 1e-6
KQ_EPS = 1e-6
NCORE = 8
HPC = H // NCORE          # heads per core (2)
TPC = B * S // NCORE      # tokens per core (512)
NQC = S // 512            # q-chunks per batch (4)
BOFF = 1024               # bias vec offset: index j = d + BOFF
MASKVAL = -20.0


def rel_bucket(d):
    d = np.asarray(d)
    max_exact = NB // 2
    safe = np.maximum(d, 1).astype(np.float64)
    large = max_exact + (
        np.log(safe / max_exact) / math.log(MAXD / max_exact) * (NB - max_exact)
    ).astype(np.int32)
    large = np.minimum(large, NB - 1)
    return np.where(d < max_exact, d, large)


def build_launch_a():
    nc = bacc.Bacc("TRN2", target_bir_lowering=False, debug=False)
    xT_d = nc.dram_tensor("xT", [E, B * S], F32R, kind="ExternalInput").ap()
    wq_d = nc.dram_tensor("wq", [128, 8 * HPC * D], F32R, kind="ExternalInput").ap()
    wk_d = nc.dram_tensor("wk", [128, 8 * HPC * D], F32R, kind="ExternalInput").ap()
    wv_d = nc.dram_tensor("wv", [128, 8 * HPC * D], F32R, kind="ExternalInput").ap()
    biast_d = nc.dram_tensor("biast", [HPC * 5, 128, 512], F32,
                             kind="ExternalInput")
    fconst_d = nc.dram_tensor("fconst", [128, HPC], F32, kind="ExternalInput").ap()
    scale2_d = nc.dram_tensor("scale2", [HPC, 1], F32, kind="ExternalInput").ap()
    hsel_d = nc.dram_tensor("hsel", [HPC, 128], F32R, kind="ExternalInput").ap()
    onesc_d = nc.dram_tensor("onesc", [128, 1], F32R, kind="ExternalInput").ap()
    onesr_d = nc.dram_tensor("onesr", [1, 128], F32R, kind="ExternalInput").ap()
    ones64_d = nc.dram_tensor("ones64", [1, 64], F32R, kind="ExternalInput").ap()
    hsum_d = nc.dram_tensor("hsum", [128, HPC], F32R, kind="ExternalInput").ap()
    identc_d = nc.dram_tensor("identc", [128, 64], F32R, kind="ExternalInput").ap()
    oT_d = nc.dram_tensor("oT", [HPC * D, B * S], F32, kind="ExternalOutput").ap()

    with tile.TileContext(nc) as tc:
        with nc.allow_low_precision(reason="fp32r kernel"), \
             tc.tile_pool(name="const", bufs=1) as cpool, \
             tc.tile_pool(name="xt", bufs=1) as xpool, \
             tc.tile_pool(name="w", bufs=1) as wpool, \
             tc.tile_pool(name="qkv", bufs=1) as qkvpool, \
             tc.tile_pool(name="rawp", bufs=2) as rawpool, \
             tc.tile_pool(name="vtok", bufs=1) as vpool, \
             tc.tile_pool(name="ptile", bufs=4) as ppool, \
             tc.tile_pool(name="onorm", bufs=3) as opool, \
             tc.tile_pool(name="small", bufs=2) as spool, \
             tc.tile_pool(name="r1p", bufs=1) as rpool, \
             tc.tile_pool(name="ps_mm", bufs=4, space="PSUM") as ps_mm, \
             tc.tile_pool(name="ps_acc", bufs=2, space="PSUM") as ps_acc:

            # ---- constants ----
            fconst_t = cpool.tile([128, HPC], F32)
            nc.sync.dma_start(fconst_t[:], fconst_d[:])
            scale2_t = cpool.tile([HPC, 1], F32)
            nc.sync.dma_start(scale2_t[:], scale2_d[:])
            ones128 = cpool.tile([128, 1], F32R)
            nc.sync.dma_start(ones128[:], onesc_d[:])
            ones1x64 = cpool.tile([1, 64], F32R)
            nc.sync.dma_start(ones1x64[:], ones64_d[:])
            onesr = cpool.tile([1, 128], F32R)
            nc.sync.dma_start(onesr[:], onesr_d[:])
            hsum = cpool.tile([128, HPC], F32R)
            nc.sync.dma_start(hsum[:], hsum_d[:])
            # head-select [2, 128]: row h = 1 on cols h*64..
            hsel = cpool.tile([HPC, 128], F32R)
            nc.sync.dma_start(hsel[:], hsel_d[:])
            # stacked identities [128, 64] (f32r) for per-head PE transpose
            epsln_t = cpool.tile([128, 1], F32)
            nc.vector.memset(epsln_t[:], LN_EPS)
            epskq_t = cpool.tile([128, 1], F32)
            nc.vector.memset(epskq_t[:], KQ_EPS)
            ident = cpool.tile([128, 64], F32R)
            nc.sync.dma_start(ident[:], identc_d[:])
            # near-band bias tiles: [128, (h*5+di)*512 + f]
            biast = cpool.tile([128, HPC * 5 * 512], F32)
            nc.sync.dma_start(
                biast[:],
                bass_rust.AP(biast_d, 0, [[512, 128], [65536, HPC * 5],
                                          [1, 512]]))

            # weights (ln1-folded, f32r): [128, e-tile, col]
            wq_t = wpool.tile([128, 8, HPC * D], F32R, tag="wq")
            wk_t = wpool.tile([128, 8, HPC * D], F32R, tag="wk")
            wv_t = wpool.tile([128, 8, HPC * D], F32R, tag="wv")
            for (w_t, w_d) in ((wq_t, wq_d), (wk_t, wk_d), (wv_t, wv_d)):
                nc.sync.dma_start(
                    w_t[:], w_d.rearrange("p (i m) -> p i m", i=8))

            for b in range(B):
                # ---- xT e-tiles [128, 2048] ----
                xts = []
                for i in range(8):
                    xt = xpool.tile([128, S], F32R, tag=f"x{i}")
                    nc.sync.dma_start(xt[:], xT_d[i * 128:(i + 1) * 128,
                                                  b * S:(b + 1) * S])
                    xts.append(xt)

                # ---- r1 = 1/sqrt(mean(x^2)+eps) ----
                r1 = rpool.tile([1, S], F32R, tag="r1")
                for ch in range(S // 512):
                    csl = slice(ch * 512, (ch + 1) * 512)
                    ssx_ps = ps_acc.tile([128, 512], F32, tag="acc")
                    for i in range(8):
                        sq = spool.tile([128, 512], F32R, tag="sqx")
                        nc.scalar.activation(sq[:], xts[i][:, csl], AF.Square)
                        nc.tensor.matmul(ssx_ps[0:1, :], ones128[:], sq[:],
                                         start=(i == 0), stop=(i == 7))
                    r1sq = spool.tile([1, 512], F32, tag="r1sq")
                    nc.scalar.activation(r1sq[:], ssx_ps[0:1, :], AF.Sqrt,
                                         bias=epsln_t[0:1, :], scale=1.0 / E)
                    nc.vector.reciprocal(r1[:, csl], r1sq[:])

                # ---- projections: raw qT/kT/vT packed [128, 2048] ----
                def project(w_t, name):
                    pool_ = rawpool if name == "raw" else qkvpool
                    out = pool_.tile([128, S], F32R, tag=name)
                    for ch in range(S // 512):
                        csl = slice(ch * 512, (ch + 1) * 512)
                        ps = ps_mm.tile([128, 512], F32, tag="mm")
                        for i in range(8):
                            nc.tensor.matmul(
                                ps[:], w_t[:, i, :], xts[i][:, csl],
                                start=(i == 0), stop=(i == 7))
                        nc.scalar.activation(out[:, csl], ps[:], AF.Copy)
                    return out

                qT_raw = project(wq_t, "raw")
                kT_raw = project(wk_t, "raw")
                vT_raw = project(wv_t, "vT_raw")

                # ---- L2-normalize q (scale-folded) and k ----
                def l2norm(raw, name, fold_scale):
                    out = qkvpool.tile([128, S], F32R, tag=name)
                    for ch in range(S // 512):
                        csl = slice(ch * 512, (ch + 1) * 512)
                        sq = spool.tile([128, 512], F32R, tag="sqn")
                        nc.scalar.activation(sq[:], raw[:, csl], AF.Square)
                        ss_ps = ps_mm.tile([128, 512], F32, tag="mm")
                        nc.tensor.matmul(ss_ps[0:HPC, :], hsum[:], sq[:],
                                         start=True, stop=True)
                        rnsq = spool.tile([HPC, 512], F32, tag="rnsq")
                        nc.scalar.activation(rnsq[:], ss_ps[0:HPC, :], AF.Sqrt,
                                             bias=epskq_t[0:HPC, :])
                        rn = spool.tile([HPC, 512], F32R, tag="rn")
                        nc.vector.reciprocal(rn[:], rnsq[:])
                        if fold_scale:
                            nc.vector.tensor_scalar_mul(rn[:], rn[:],
                                                        scale2_t[:])
                        bc_ps = ps_mm.tile([128, 512], F32, tag="mm")
                        nc.tensor.matmul(bc_ps[:], hsel[:], rn[:],
                                         start=True, stop=True)
                        nc.vector.tensor_mul(out[:, csl], raw[:, csl], bc_ps[:])
                    return out

                qT = l2norm(qT_raw, "qT", True)
                kT = l2norm(kT_raw, "kT", False)

                # ---- v scaled by r1 in place (feature-major) ----
                vT_s = vT_raw
                mvv = spool.tile([128, 1], F32, tag="mvv")
                mv4 = spool.tile([128, NQC], F32, tag="mv4")
                for ch in range(NQC):
                    csl = slice(ch * 512, (ch + 1) * 512)
                    bc_ps = ps_mm.tile([128, 512], F32, tag="mm")
                    nc.tensor.matmul(bc_ps[:], onesr[:], r1[:, csl],
                                     start=True, stop=True)
                    nc.vector.tensor_mul(vT_s[:, csl], vT_raw[:, csl], bc_ps[:])
                    nc.vector.reduce_sum(mv4[:, ch:ch + 1], vT_s[:, csl],
                                         mybir.AxisListType.X)
                nc.vector.reduce_sum(mvv[:], mv4[:], mybir.AxisListType.X)

                v_augs = [[], []]
                for kt in range(S // 128):
                    for h in range(HPC):
                        va = vpool.tile([128, D + 1], F32R, tag=f"va{h}_{kt}")
                        nc.vector.tensor_copy(va[:, D:D + 1], ones128[:])
                        tr_ps = ps_mm.tile([128, 512], F32R, tag="mm")
                        nc.tensor.transpose(
                            tr_ps[0:128, 0:64],
                            vT_s[h * D:(h + 1) * D, kt * 128:(kt + 1) * 128],
                            ident[h * D:(h + 1) * D, :])
                        nc.vector.tensor_copy(va[:, 0:D], tr_ps[0:128, 0:64])
                        v_augs[h].append(va)

                # ---- attention ----
                for h in range(HPC):
                    hd = slice(h * D, (h + 1) * D)
                    for qc in range(NQC):
                        Q0 = qc * 512
                        qsl = slice(Q0, Q0 + 512)
                        nkt = (Q0 + 512) // 128
                        o_ps = ps_acc.tile([128, 512], F32, tag="acc")
                        for kt in range(nkt):
                            K0 = kt * 128
                            s_ps = ps_mm.tile([128, 512], F32, tag="mm")
                            nc.tensor.matmul(
                                s_ps[:], kT[hd, K0:K0 + 128], qT[hd, qsl],
                                start=True, stop=True)
                            p_t = ppool.tile([128, 512], F32R, tag="p")
                            if Q0 - K0 >= 255:
                                nc.scalar.activation(
                                    p_t[:], s_ps[:], AF.Exp,
                                    bias=fconst_t[:, h:h + 1])
                            else:
                                di = 1 + kt - 4 * qc
                                bsl = slice((h * 5 + di) * 512,
                                            (h * 5 + di + 1) * 512)
                                nc.vector.tensor_add(s_ps[:], s_ps[:],
                                                     biast[:, bsl])
                                nc.scalar.activation(p_t[:], s_ps[:], AF.Exp)
                            nc.tensor.matmul(
                                o_ps[0:D + 1, :], v_augs[h][kt][:], p_t[:],
                                start=(kt == 0), stop=(kt == nkt - 1))
                        srec = spool.tile([1, 512], F32R, tag="srec")
                        nc.vector.reciprocal(srec[:], o_ps[D:D + 1, :])
                        nb_ps = ps_mm.tile([128, 512], F32, tag="mm")
                        nc.tensor.matmul(nb_ps[0:D, :], ones1x64[:], srec[:],
                                         start=True, stop=True)
                        nb = spool.tile([64, 512], F32, tag="nbs")
                        nc.scalar.activation(nb[:], nb_ps[0:D, :], AF.Copy)
                        o_n = opool.tile([64, 512], F32, tag="on")
                        nc.vector.tensor_mul(o_n[:], o_ps[0:D, :], nb[:])
                        if qc == 0:
                            nc.scalar.activation(
                                o_n[:, 0:1], mvv[h * D:(h + 1) * D, :],
                                AF.Copy, scale=1.0 / S)
                        nc.sync.dma_start(
                            oT_d[hd, b * S + Q0: b * S + Q0 + 512], o_n[:])
    nc.compile()
    return nc


def prep_a_inputs(inputs):
    x = np.asarray(inputs["x"], np.float32)
    ln1 = np.asarray(inputs["ln1_w"], np.float32)
    Wq = np.asarray(inputs["Wq"], np.float32)
    Wk = np.asarray(inputs["Wk"], np.float32)
    Wv = np.asarray(inputs["Wv"], np.float32)
    rb = np.asarray(inputs["rel_bias"], np.float32)
    scale = np.asarray(inputs["scale"], np.float32)
    xT = np.ascontiguousarray(x.reshape(B * S, E).T)
    d = np.arange(2048) - BOFF
    bucket = rel_bucket(np.maximum(d, 1))
    biasv_all = np.where(
        (d < 1)[None, :], np.float32(MASKVAL),
        scale[:, None] * rb[bucket, :].T.astype(np.float32)).astype(np.float32)
    # bias delta-tiles: [H, 5, 128, 512]; B[h,di,p,f] = v_h[BOFF+128-128*di+f-p]
    di_ = np.arange(5)[:, None, None]
    p_ = np.arange(128)[None, :, None]
    f_ = np.arange(512)[None, None, :]
    idx = BOFF + 128 - 128 * di_ + f_ - p_
    biast_all = biasv_all[:, idx]  # [H, 5, 128, 512]
    fconst_all = (scale * rb[NB - 1, :]).astype(np.float32)

    def tile_w(w):  # [1024, M] -> [128, 8*M]
        M = w.shape[1]
        return np.ascontiguousarray(
            w.reshape(8, 128, M).transpose(1, 0, 2).reshape(128, 8 * M))
    in_maps = []
    for c in range(NCORE):
        hs = slice(c * HPC, (c + 1) * HPC)
        cs = slice(c * HPC * D, (c + 1) * HPC * D)
        hsel_np = np.zeros((HPC, 128), np.float32)
        for h in range(HPC):
            hsel_np[h, h * D:(h + 1) * D] = 1.0
        ident_np = np.concatenate([np.eye(D, dtype=np.float32)] * 2, axis=0)
        hsum_np = np.zeros((128, HPC), np.float32)
        for h in range(HPC):
            hsum_np[h * D:(h + 1) * D, h] = 1.0
        in_maps.append({
            "xT": xT,
            "hsel": hsel_np,
            "identc": ident_np,
            "onesc": np.ones((128, 1), np.float32),
            "onesr": np.ones((1, 128), np.float32),
            "ones64": np.ones((1, 64), np.float32),
            "hsum": hsum_np,
            "wq": tile_w(ln1[:, None] * Wq[:, cs]),
            "wk": tile_w(ln1[:, None] * Wk[:, cs]),
            "wv": tile_w(ln1[:, None] * Wv[:, cs]),
            "biast": np.ascontiguousarray(
                biast_all[hs].reshape(HPC * 5, 128, 512)),
            "fconst": np.ascontiguousarray(
                np.broadcast_to(fconst_all[hs], (128, HPC))),
            "scale2": np.ascontiguousarray(scale[hs, None]),
        })
    return in_maps


def build_launch_b():
    nc = bacc.Bacc("TRN2", target_bir_lowering=False, debug=False)
    oT_d = nc.dram_tensor("oTs", [E, TPC], F32R, kind="ExternalInput").ap()
    xT_d = nc.dram_tensor("xTs", [E, TPC], F32R, kind="ExternalInput").ap()
    wo_d = nc.dram_tensor("wo", [E // 128, 128, E], F32R, kind="ExternalInput")
    w1_d = nc.dram_tensor("w1", [HID // 128, 128, E], F32R, kind="ExternalInput")
    w2_d = nc.dram_tensor("w2", [E // 128, 128, HID], F32R, kind="ExternalInput")
    onesc_d = nc.dram_tensor("onesc", [128, 1], F32R, kind="ExternalInput").ap()
    onesr_d = nc.dram_tensor("onesr", [1, 128], F32R, kind="ExternalInput").ap()
    out_d = nc.dram_tensor("outT", [E, TPC], F32, kind="ExternalOutput").ap()

    NE = E // 128    # 8 e-tiles
    NH = HID // 128  # 32 h-tiles

    with tile.TileContext(nc) as tc:
        with nc.allow_low_precision(reason="fp32r kernel"), \
             tc.tile_pool(name="const", bufs=1) as cpool, \
             tc.tile_pool(name="io", bufs=1) as iopool, \
             tc.tile_pool(name="y", bufs=1) as ypool, \
             tc.tile_pool(name="h", bufs=1) as hpool, \
             tc.tile_pool(name="w1s", bufs=2) as w1pool, \
             tc.tile_pool(name="w2s", bufs=2) as w2pool, \
             tc.tile_pool(name="small", bufs=2) as spool, \
             tc.tile_pool(name="r1p", bufs=1) as rpool, \
             tc.tile_pool(name="ps_mm", bufs=4, space="PSUM") as ps_mm, \
             tc.tile_pool(name="ps_acc", bufs=2, space="PSUM") as ps_acc:

            ones128 = cpool.tile([128, 1], F32R)
            nc.sync.dma_start(ones128[:], onesc_d[:])
            epsln_t = cpool.tile([128, 1], F32)
            nc.vector.memset(epsln_t[:], LN_EPS)

            oTs, xTs = [], []
            for i in range(NE):
                ot = iopool.tile([128, TPC], F32R, tag=f"o{i}")
                nc.sync.dma_start(ot[:], oT_d[i * 128:(i + 1) * 128, :])
                oTs.append(ot)
                xt = iopool.tile([128, TPC], F32R, tag=f"xs{i}")
                nc.sync.dma_start(xt[:], xT_d[i * 128:(i + 1) * 128, :])
                xTs.append(xt)
            # ---- y^T = Wo^T @ O^T + x^T ----
            yTs = []
            for i in range(NE):
                wo_t = w1pool.tile([128, NE, 128], F32R, tag="w1")
                nc.sync.dma_start(
                    wo_t[:], bass_rust.AP(wo_d, i * 128 * E,
                                          [[E, 128], [1, NE * 128]]))
                ps = ps_acc.tile([128, TPC], F32, tag="acc")
                for j in range(NE):
                    nc.tensor.matmul(ps[:], wo_t[:, j, :],
                                     oTs[j][:], start=(j == 0), stop=(j == NE - 1))
                yt = ypool.tile([128, TPC], F32R, tag=f"y{i}")
                nc.vector.tensor_add(yt[:], ps[:], xTs[i][:])
                yTs.append(yt)

            # ---- r2 = 1/sqrt(mean(y^2)+eps) ----
            ssy_ps = ps_acc.tile([128, TPC], F32, tag="acc")
            for i in range(NE):
                sq = spool.tile([128, TPC], F32R, tag="sqy")
                nc.scalar.activation(sq[:], yTs[i][:], AF.Square)
                nc.tensor.matmul(ssy_ps[0:1, :], ones128[:], sq[:],
                                 start=(i == 0), stop=(i == NE - 1))
            r2sq = spool.tile([1, TPC], F32, tag="r2sq")
            nc.scalar.activation(r2sq[:], ssy_ps[0:1, :], AF.Sqrt,
                                 bias=epsln_t[0:1, :], scale=1.0 / E)
            r2 = spool.tile([1, TPC], F32R, tag="r2")
            nc.vector.reciprocal(r2[:], r2sq[:])
            # broadcast r2 over 128 partitions
            r2b_ps = ps_mm.tile([128, TPC], F32, tag="mm")
            o1x128 = cpool.tile([1, 128], F32R)
            nc.sync.dma_start(o1x128[:], onesr_d[:])
            nc.tensor.matmul(r2b_ps[:], o1x128[:], r2[:], start=True, stop=True)
            r2b = cpool.tile([128, TPC], F32)
            nc.scalar.activation(r2b[:], r2b_ps[:], AF.Copy)

            # ---- h^T = relu(W1'^T y^T) ----
            hts = []
            for ht in range(NH):
                w1_t = w1pool.tile([128, NE, 128], F32R, tag="w1")
                nc.sync.dma_start(
                    w1_t[:], bass_rust.AP(w1_d, ht * 128 * E,
                                          [[E, 128], [1, NE * 128]]))
                ps = ps_mm.tile([128, TPC], F32, tag="mm")
                for j in range(NE):
                    nc.tensor.matmul(ps[:], w1_t[:, j, :], yTs[j][:],
                                     start=(j == 0), stop=(j == NE - 1))
                h_t = hpool.tile([128, TPC], F32R, tag=f"h{ht}")
                nc.scalar.activation(h_t[:], ps[:], AF.Relu)
                hts.append(h_t)

            # ---- z = (h^T' W2)^T * r2 + y ----
            for i in range(NE):
                w2_t = w2pool.tile([128, NH, 128], F32R, tag="w2")
                nc.sync.dma_start(
                    w2_t[:], bass_rust.AP(w2_d, i * 128 * HID,
                                          [[HID, 128], [1, NH * 128]]))
                ps = ps_acc.tile([128, TPC], F32, tag="acc")
                for ht in range(NH):
                    nc.tensor.matmul(ps[:], w2_t[:, ht, :], hts[ht][:],
                                     start=(ht == 0), stop=(ht == NH - 1))
                zt = spool.tile([128, TPC], F32, tag="zt")
                nc.vector.tensor_mul(zt[:], ps[:], r2b[:])
                outt = spool.tile([128, TPC], F32, tag="outt")
                nc.vector.tensor_add(outt[:], zt[:], yTs[i][:])
                nc.sync.dma_start(out_d[i * 128:(i + 1) * 128, :], outt[:])
    nc.compile()
    return nc


def prep_b_inputs(inputs, oT_all):
    x = np.asarray(inputs["x"], np.float32)
    ln2 = np.asarray(inputs["ln2_w"], np.float32)
    def tile_cols(w):
        # [K, M] -> [M//128, 128, K]: out[i, p, j*128+m] = w[j*128+p, i*128+m]
        K, M = w.shape
        return np.ascontiguousarray(
            w.reshape(K // 128, 128, M // 128, 128)
            .transpose(2, 1, 0, 3).reshape(M // 128, 128, K))
    Wo = tile_cols(np.asarray(inputs["Wo"], np.float32))
    W1 = tile_cols(ln2[:, None] * np.asarray(inputs["W1"], np.float32))
    W2 = tile_cols(np.asarray(inputs["W2"], np.float32))
    xT = x.reshape(B * S, E).T
    in_maps = []
    for c in range(NCORE):
        ts = slice(c * TPC, (c + 1) * TPC)
        in_maps.append({
            "oTs": np.ascontiguousarray(oT_all[:, ts]),
            "onesc": np.ones((128, 1), np.float32),
            "onesr": np.ones((1, 128), np.float32),
            "xTs": np.ascontiguousarray(xT[:, ts]),
            "wo": Wo, "w1": W1, "w2": W2,
        })
    return in_maps


_CACHE = {}


def _get_compiled():
    if "a" not in _CACHE:
        _CACHE["a"] = build_launch_a()
    if "b" not in _CACHE:
        _CACHE["b"] = build_launch_b()
    return _CACHE["a"], _CACHE["b"]


def kernel(**inputs):
    from concourse import bass_utils
    inputs = {k: np.asarray(v) for k, v in inputs.items()}
    nca, ncb = _get_compiled()
    in_maps_a = prep_a_inputs(inputs)
    res_a = bass_utils.run_bass_kernel_spmd(
        nca, in_maps_a, core_ids=list(range(NCORE)))
    oT_all = np.concatenate([res_a.results[c]["oT"] for c in range(NCORE)],
                            axis=0)  # [E, B*S], rows = h*64+d
    in_maps_b = prep_b_inputs(inputs, oT_all)
    res_b = bass_utils.run_bass_kernel_spmd(
        ncb, in_maps_b, core_ids=list(range(NCORE)))
    outT = np.concatenate([res_b.results[c]["outT"] for c in range(NCORE)],
                          axis=1)  # [E, B*S]
    return np.ascontiguousarray(outT.T).reshape(B, S, E).astype(np.float32)



# revision 5
# speedup vs baseline: 1.1493x; 1.1493x over previous
# Trainium2 Bass kernels for nn_DecoderLayer (T5-style decoder layer).
# Two SPMD launches over 8 cores:
#   A: head-parallel attention (2 heads/core) -> per-head attn outputs O^T
#   B: token-parallel output-proj + FFN (512 tokens/core)
# Activations kept feature-major (features on partitions).
import sys
sys.path.insert(0, '/opt/trn_rl_repo')
import math
import numpy as np
import bass_rust
import concourse.bass as bass
import concourse.mybir as mybir
import concourse.tile as tile
from concourse import bacc

F32 = mybir.dt.float32
F32R = mybir.dt.float32r
F8 = mybir.dt.float8e4
BF16 = mybir.dt.bfloat16
AF = mybir.ActivationFunctionType
DRMODE = mybir.MatmulPerfMode.DoubleRow
import ml_dtypes
FP8NP = ml_dtypes.float8_e4m3

E = 1024
H = 16
D = 64
HID = 4096
B = 2
S = 2048
NB = 32
MAXD = 128
LN_EPS = 1e-6
KQ_EPS = 1e-6
NCORE = 8
HPC = H // NCORE          # heads per core (2)
TPC = B * S // NCORE      # tokens per core (512)
NQC = S // 512            # q-chunks per batch (4)
BOFF = 1024               # bias vec offset: index j = d + BOFF
MASKVAL = -20.0


def rel_bucket(d):
    d = np.asarray(d)
    max_exact = NB // 2
    safe = np.maximum(d, 1).astype(np.float64)
    large = max_exact + (
        np.log(safe / max_exact) / math.log(MAXD / max_exact) * (NB - max_exact)
    ).astype(np.int32)
    large = np.minimum(large, NB - 1)
    return np.where(d < max_exact, d, large)


def build_launch_a():
    nc = bacc.Bacc("TRN2", target_bir_lowering=False, debug=False)
    xT_d = nc.dram_tensor("xT", [E, B * S], F32R, kind="ExternalInput").ap()
    wq_d = nc.dram_tensor("wq", [128, 8 * HPC * D], F32R, kind="ExternalInput").ap()
    wk_d = nc.dram_tensor("wk", [128, 8 * HPC * D], F32R, kind="ExternalInput").ap()
    wv_d = nc.dram_tensor("wv", [128, 8 * HPC * D], F32R, kind="ExternalInput").ap()
    biast_d = nc.dram_tensor("biast", [HPC * 5, 128, 512], F32,
                             kind="ExternalInput")
    fconst_d = nc.dram_tensor("fconst", [128, HPC], F32, kind="ExternalInput").ap()
    scale2_d = nc.dram_tensor("scale2", [HPC, 1], F32, kind="ExternalInput").ap()
    hsel_d = nc.dram_tensor("hsel", [HPC, 128], F32R, kind="ExternalInput").ap()
    onesc_d = nc.dram_tensor("onesc", [128, 1], F32R, kind="ExternalInput").ap()
    onesr_d = nc.dram_tensor("onesr", [1, 128], F32R, kind="ExternalInput").ap()
    ones64_d = nc.dram_tensor("ones64", [1, 64], F32R, kind="ExternalInput").ap()
    hsum_d = nc.dram_tensor("hsum", [128, HPC], F32R, kind="ExternalInput").ap()
    identc_d = nc.dram_tensor("identc", [128, 64], F32R, kind="ExternalInput").ap()
    oT_d = nc.dram_tensor("oT", [HPC * D, B * S], F32, kind="ExternalOutput").ap()

    with tile.TileContext(nc) as tc:
        with nc.allow_low_precision(reason="fp32r kernel"), \
             tc.tile_pool(name="const", bufs=1) as cpool, \
             tc.tile_pool(name="xt", bufs=1) as xpool, \
             tc.tile_pool(name="w", bufs=1) as wpool, \
             tc.tile_pool(name="qkv", bufs=1) as qkvpool, \
             tc.tile_pool(name="rawp", bufs=2) as rawpool, \
             tc.tile_pool(name="vtok", bufs=1) as vpool, \
             tc.tile_pool(name="ptile", bufs=4) as ppool, \
             tc.tile_pool(name="onorm", bufs=3) as opool, \
             tc.tile_pool(name="small", bufs=2) as spool, \
             tc.tile_pool(name="r1p", bufs=1) as rpool, \
             tc.tile_pool(name="ps_mm", bufs=4, space="PSUM") as ps_mm, \
             tc.tile_pool(name="ps_acc", bufs=2, space="PSUM") as ps_acc:

            # ---- constants ----
            fconst_t = cpool.tile([128, HPC], F32)
            nc.sync.dma_start(fconst_t[:], fconst_d[:])
            scale2_t = cpool.tile([HPC, 1], F32)
            nc.sync.dma_start(scale2_t[:], scale2_d[:])
            ones128 = cpool.tile([128, 1], F32R)
            nc.sync.dma_start(ones128[:], onesc_d[:])
            ones1x64 = cpool.tile([1, 64], F32R)
            nc.sync.dma_start(ones1x64[:], ones64_d[:])
            onesr = cpool.tile([1, 128], F32R)
            nc.sync.dma_start(onesr[:], onesr_d[:])
            hsum = cpool.tile([128, HPC], F32R)
            nc.sync.dma_start(hsum[:], hsum_d[:])
            # head-select [2, 128]: row h = 1 on cols h*64..
            hsel = cpool.tile([HPC, 128], F32R)
            nc.sync.dma_start(hsel[:], hsel_d[:])
            # stacked identities [128, 64] (f32r) for per-head PE transpose
            epsln_t = cpool.tile([128, 1], F32)
            nc.vector.memset(epsln_t[:], LN_EPS)
            epskq_t = cpool.tile([128, 1], F32)
            nc.vector.memset(epskq_t[:], KQ_EPS)
            ident = cpool.tile([128, 64], F32R)
            nc.sync.dma_start(ident[:], identc_d[:])
            # near-band bias tiles: [128, (h*5+di)*512 + f]
            biast = cpool.tile([128, HPC * 5 * 512], F32)
            nc.sync.dma_start(
                biast[:],
                bass_rust.AP(biast_d, 0, [[512, 128], [65536, HPC * 5],
                                          [1, 512]]))

            # weights (ln1-folded, f32r): [128, e-tile, col]
            wq_t = wpool.tile([128, 8, HPC * D], F32R, tag="wq")
            wk_t = wpool.tile([128, 8, HPC * D], F32R, tag="wk")
            wv_t = wpool.tile([128, 8, HPC * D], F32R, tag="wv")
            for (w_t, w_d) in ((wq_t, wq_d), (wk_t, wk_d), (wv_t, wv_d)):
                nc.sync.dma_start(
                    w_t[:], w_d.rearrange("p (i m) -> p i m", i=8))

            for b in range(B):
                # ---- xT e-tiles [128, 2048] ----
                xts = []
                for i in range(8):
                    xt = xpool.tile([128, S], F32R, tag=f"x{i}")
                    nc.sync.dma_start(xt[:], xT_d[i * 128:(i + 1) * 128,
                                                  b * S:(b + 1) * S])
                    xts.append(xt)

                # ---- r1 = 1/sqrt(mean(x^2)+eps) ----
                r1 = rpool.tile([1, S], F32R, tag="r1")
                for ch in range(S // 512):
                    csl = slice(ch * 512, (ch + 1) * 512)
                    ssx_ps = ps_acc.tile([128, 512], F32, tag="acc")
                    for i in range(8):
                        sq = spool.tile([128, 512], F32R, tag="sqx")
                        nc.scalar.activation(sq[:], xts[i][:, csl], AF.Square)
                        nc.tensor.matmul(ssx_ps[0:1, :], ones128[:], sq[:],
                                         start=(i == 0), stop=(i == 7))
                    r1sq = spool.tile([1, 512], F32, tag="r1sq")
                    nc.scalar.activation(r1sq[:], ssx_ps[0:1, :], AF.Sqrt,
                                         bias=epsln_t[0:1, :], scale=1.0 / E)
                    nc.vector.reciprocal(r1[:, csl], r1sq[:])

                # ---- projections: raw qT/kT/vT packed [128, 2048] ----
                def project(w_t, name):
                    pool_ = rawpool if name == "raw" else qkvpool
                    out = pool_.tile([128, S], F32R, tag=name)
                    for ch in range(S // 512):
                        csl = slice(ch * 512, (ch + 1) * 512)
                        ps = ps_mm.tile([128, 512], F32, tag="mm")
                        for i in range(8):
                            nc.tensor.matmul(
                                ps[:], w_t[:, i, :], xts[i][:, csl],
                                start=(i == 0), stop=(i == 7))
                        nc.scalar.activation(out[:, csl], ps[:], AF.Copy)
                    return out

                qT_raw = project(wq_t, "raw")
                kT_raw = project(wk_t, "raw")
                vT_raw = project(wv_t, "vT_raw")

                # ---- L2-normalize q (scale-folded) and k ----
                def l2norm(raw, name, fold_scale):
                    out = qkvpool.tile([128, S], F32R, tag=name)
                    for ch in range(S // 512):
                        csl = slice(ch * 512, (ch + 1) * 512)
                        sq = spool.tile([128, 512], F32R, tag="sqn")
                        nc.scalar.activation(sq[:], raw[:, csl], AF.Square)
                        ss_ps = ps_mm.tile([128, 512], F32, tag="mm")
                        nc.tensor.matmul(ss_ps[0:HPC, :], hsum[:], sq[:],
                                         start=True, stop=True)
                        rnsq = spool.tile([HPC, 512], F32, tag="rnsq")
                        nc.scalar.activation(rnsq[:], ss_ps[0:HPC, :], AF.Sqrt,
                                             bias=epskq_t[0:HPC, :])
                        rn = spool.tile([HPC, 512], F32R, tag="rn")
                        nc.vector.reciprocal(rn[:], rnsq[:])
                        if fold_scale:
                            nc.vector.tensor_scalar_mul(rn[:], rn[:],
                                                        scale2_t[:])
                        bc_ps = ps_mm.tile([128, 512], F32, tag="mm")
                        nc.tensor.matmul(bc_ps[:], hsel[:], rn[:],
                                         start=True, stop=True)
                        nc.vector.tensor_mul(out[:, csl], raw[:, csl], bc_ps[:])
                    return out

                qT = l2norm(qT_raw, "qT", True)
                kT = l2norm(kT_raw, "kT", False)

                # ---- v scaled by r1 in place (feature-major) ----
                vT_s = vT_raw
                mvv = spool.tile([128, 1], F32, tag="mvv")
                mv4 = spool.tile([128, NQC], F32, tag="mv4")
                for ch in range(NQC):
                    csl = slice(ch * 512, (ch + 1) * 512)
                    bc_ps = ps_mm.tile([128, 512], F32, tag="mm")
                    nc.tensor.matmul(bc_ps[:], onesr[:], r1[:, csl],
                                     start=True, stop=True)
                    nc.vector.tensor_mul(vT_s[:, csl], vT_raw[:, csl], bc_ps[:])
                    nc.vector.reduce_sum(mv4[:, ch:ch + 1], vT_s[:, csl],
                                         mybir.AxisListType.X)
                nc.vector.reduce_sum(mvv[:], mv4[:], mybir.AxisListType.X)

                v_augs = [[], []]
                for kt in range(S // 128):
                    for h in range(HPC):
                        va = vpool.tile([128, D + 1], F32R, tag=f"va{h}_{kt}")
                        nc.vector.tensor_copy(va[:, D:D + 1], ones128[:])
                        tr_ps = ps_mm.tile([128, 512], F32R, tag="mm")
                        nc.tensor.transpose(
                            tr_ps[0:128, 0:64],
                            vT_s[h * D:(h + 1) * D, kt * 128:(kt + 1) * 128],
                            ident[h * D:(h + 1) * D, :])
                        nc.vector.tensor_copy(va[:, 0:D], tr_ps[0:128, 0:64])
                        v_augs[h].append(va)

                # ---- attention ----
                for h in range(HPC):
                    hd = slice(h * D, (h + 1) * D)
                    for qc in range(NQC):
                        Q0 = qc * 512
                        qsl = slice(Q0, Q0 + 512)
                        nkt = (Q0 + 512) // 128
                        o_ps = ps_acc.tile([128, 512], F32, tag="acc")
                        for kt in range(nkt):
                            K0 = kt * 128
                            s_ps = ps_mm.tile([128, 512], F32, tag="mm")
                            nc.tensor.matmul(
                                s_ps[:], kT[hd, K0:K0 + 128], qT[hd, qsl],
                                start=True, stop=True)
                            p_t = ppool.tile([128, 512], F32R, tag="p")
                            if Q0 - K0 >= 255:
                                nc.scalar.activation(
                                    p_t[:], s_ps[:], AF.Exp,
                                    bias=fconst_t[:, h:h + 1])
                            else:
                                di = 1 + kt - 4 * qc
                                bsl = slice((h * 5 + di) * 512,
                                            (h * 5 + di + 1) * 512)
                                nc.vector.tensor_add(s_ps[:], s_ps[:],
                                                     biast[:, bsl])
                                nc.scalar.activation(p_t[:], s_ps[:], AF.Exp)
                            nc.tensor.matmul(
                                o_ps[0:D + 1, :], v_augs[h][kt][:], p_t[:],
                                start=(kt == 0), stop=(kt == nkt - 1))
                        srec = spool.tile([1, 512], F32R, tag="srec")
                        nc.vector.reciprocal(srec[:], o_ps[D:D + 1, :])
                        nb_ps = ps_mm.tile([128, 512], F32, tag="mm")
                        nc.tensor.matmul(nb_ps[0:D, :], ones1x64[:], srec[:],
                                         start=True, stop=True)
                        nb = spool.tile([64, 512], F32, tag="nbs")
                        nc.scalar.activation(nb[:], nb_ps[0:D, :], AF.Copy)
                        o_n = opool.tile([64, 512], F32, tag="on")
                        nc.vector.tensor_mul(o_n[:], o_ps[0:D, :], nb[:])
                        if qc == 0:
                            nc.scalar.activation(
                                o_n[:, 0:1], mvv[h * D:(h + 1) * D, :],
                                AF.Copy, scale=1.0 / S)
                        nc.sync.dma_start(
                            oT_d[hd, b * S + Q0: b * S + Q0 + 512], o_n[:])
    nc.compile()
    return nc


def prep_a_inputs(inputs):
    x = np.asarray(inputs["x"], np.float32)
    ln1 = np.asarray(inputs["ln1_w"], np.float32)
    Wq = np.asarray(inputs["Wq"], np.float32)
    Wk = np.asarray(inputs["Wk"], np.float32)
    Wv = np.asarray(inputs["Wv"], np.float32)
    rb = np.asarray(inputs["rel_bias"], np.float32)
    scale = np.asarray(inputs["scale"], np.float32)
    xT = np.ascontiguousarray(x.reshape(B * S, E).T)
    d = np.arange(2048) - BOFF
    bucket = rel_bucket(np.maximum(d, 1))
    biasv_all = np.where(
        (d < 1)[None, :], np.float32(MASKVAL),
        scale[:, None] * rb[bucket, :].T.astype(np.float32)).astype(np.float32)
    # bias delta-tiles: [H, 5, 128, 512]; B[h,di,p,f] = v_h[BOFF+128-128*di+f-p]
    di_ = np.arange(5)[:, None, None]
    p_ = np.arange(128)[None, :, None]
    f_ = np.arange(512)[None, None, :]
    idx = BOFF + 128 - 128 * di_ + f_ - p_
    biast_all = biasv_all[:, idx]  # [H, 5, 128, 512]
    fconst_all = (scale * rb[NB - 1, :]).astype(np.float32)

    def tile_w(w):  # [1024, M] -> [128, 8*M]
        M = w.shape[1]
        return np.ascontiguousarray(
            w.reshape(8, 128, M).transpose(1, 0, 2).reshape(128, 8 * M))
    in_maps = []
    for c in range(NCORE):
        hs = slice(c * HPC, (c + 1) * HPC)
        cs = slice(c * HPC * D, (c + 1) * HPC * D)
        hsel_np = np.zeros((HPC, 128), np.float32)
        for h in range(HPC):
            hsel_np[h, h * D:(h + 1) * D] = 1.0
        ident_np = np.concatenate([np.eye(D, dtype=np.float32)] * 2, axis=0)
        hsum_np = np.zeros((128, HPC), np.float32)
        for h in range(HPC):
            hsum_np[h * D:(h + 1) * D, h] = 1.0
        in_maps.append({
            "xT": xT,
            "hsel": hsel_np,
            "identc": ident_np,
            "onesc": np.ones((128, 1), np.float32),
            "onesr": np.ones((1, 128), np.float32),
            "ones64": np.ones((1, 64), np.float32),
            "hsum": hsum_np,
            "wq": tile_w(ln1[:, None] * Wq[:, cs]),
            "wk": tile_w(ln1[:, None] * Wk[:, cs]),
            "wv": tile_w(ln1[:, None] * Wv[:, cs]),
            "biast": np.ascontiguousarray(
                biast_all[hs].reshape(HPC * 5, 128, 512)),
            "fconst": np.ascontiguousarray(
                np.broadcast_to(fconst_all[hs], (128, HPC))),
            "scale2": np.ascontiguousarray(scale[hs, None]),
        })
    return in_maps


def build_launch_b():
    nc = bacc.Bacc("TRN2", target_bir_lowering=False, debug=False)
    NE = E // 128    # 8 e-tiles
    NH = HID // 128  # 32 h-tiles
    oT_d = nc.dram_tensor("oT16", [128, NE, TPC], BF16, kind="ExternalInput").ap()
    xT_d = nc.dram_tensor("xTs", [128, NE, TPC], F32, kind="ExternalInput").ap()
    wo_d = nc.dram_tensor("wo16", [128, NE, NE, 128], BF16,
                          kind="ExternalInput").ap()
    w1_d = nc.dram_tensor("w116", [128, NH, NE, 128], BF16,
                          kind="ExternalInput").ap()
    w2_d = nc.dram_tensor("w216", [128, NE, NH, 128], BF16,
                          kind="ExternalInput").ap()
    ones_d = nc.dram_tensor("onesb", [128, 1], BF16, kind="ExternalInput").ap()
    onesr_d = nc.dram_tensor("onesr", [1, 128], F32R, kind="ExternalInput").ap()
    out_d = nc.dram_tensor("outT", [E, TPC], F32, kind="ExternalOutput").ap()

    with tile.TileContext(nc) as tc:
        with nc.allow_low_precision(reason="bf16 kernel"), \
             tc.tile_pool(name="const", bufs=1) as cpool, \
             tc.tile_pool(name="io", bufs=1) as iopool, \
             tc.tile_pool(name="y", bufs=1) as ypool, \
             tc.tile_pool(name="h", bufs=1) as hpool, \
             tc.tile_pool(name="w1s", bufs=2) as w1pool, \
             tc.tile_pool(name="w2s", bufs=2) as w2pool, \
             tc.tile_pool(name="small", bufs=2) as spool, \
             tc.tile_pool(name="ps_mm", bufs=4, space="PSUM") as ps_mm, \
             tc.tile_pool(name="ps_acc", bufs=2, space="PSUM") as ps_acc, \
             tc.tile_pool(name="ps_r", bufs=1, space="PSUM") as ps_r:

            ones16 = cpool.tile([128, 1], BF16)
            nc.sync.dma_start(ones16[:], ones_d[:])
            o1x128 = cpool.tile([1, 128], F32R)
            nc.sync.dma_start(o1x128[:], onesr_d[:])
            epsln_t = cpool.tile([128, 1], F32)
            nc.vector.memset(epsln_t[:], LN_EPS)

            wo_t = cpool.tile([128, NE, NE, 128], BF16)
            nc.sync.dma_start(wo_t[:], wo_d[:])
            oT16 = iopool.tile([128, NE, TPC], BF16, tag="oT16")
            nc.sync.dma_start(oT16[:], oT_d[:])
            xT8 = iopool.tile([128, NE, TPC], F32, tag="xTs")
            nc.sync.dma_start(xT8[:], xT_d[:])

            # ---- y = Wo^T @ O + x ; y16 bf16 copy; sq16 squares ----
            y_t = ypool.tile([128, NE, TPC], F32, tag="y")
            y16_t = ypool.tile([128, NE, TPC], BF16, tag="y16")
            sq16_t = ypool.tile([128, NE, TPC], BF16, tag="sq16")
            for i in range(NE):
                ps = ps_mm.tile([128, TPC], F32, tag="mm")
                for j in range(NE):
                    nc.tensor.matmul(ps[:], wo_t[:, i, j, :], oT16[:, j, :],
                                     start=(j == 0), stop=(j == NE - 1))
                nc.vector.tensor_add(y_t[:, i, :], ps[:], xT8[:, i, :])
                nc.gpsimd.tensor_copy(y16_t[:, i, :], y_t[:, i, :])
                nc.gpsimd.tensor_mul(sq16_t[:, i, :], y16_t[:, i, :],
                                     y16_t[:, i, :])

            # ---- r2 = 1/sqrt(mean(y^2)+eps), broadcast to [128, TPC] ----
            ssy_ps = ps_r.tile([128, TPC], F32, tag="r")
            for i in range(NE):
                nc.tensor.matmul(ssy_ps[0:1, :], ones16[:], sq16_t[:, i, :],
                                 start=(i == 0), stop=(i == NE - 1))
            r2sq = spool.tile([1, TPC], F32, tag="r2sq")
            nc.scalar.activation(r2sq[:], ssy_ps[0:1, :], AF.Sqrt,
                                 bias=epsln_t[0:1, :], scale=1.0 / E)
            r2 = spool.tile([1, TPC], F32R, tag="r2")
            nc.vector.reciprocal(r2[:], r2sq[:])
            r2b_ps = ps_r.tile([128, TPC], F32, tag="r")
            nc.tensor.matmul(r2b_ps[:], o1x128[:], r2[:], start=True, stop=True)
            r2b = cpool.tile([128, TPC], F32)
            nc.scalar.activation(r2b[:], r2b_ps[:], AF.Copy)

            # ---- h = relu(W1'^T y) in bf16 (weights streamed in 4 groups) ----
            h16_t = hpool.tile([128, NH, TPC], BF16, tag="h16")
            G = NH // 4
            for g in range(4):
                w1_t = w1pool.tile([128, G, NE, 128], BF16, tag="w1")
                nc.sync.dma_start(w1_t[:], w1_d[:, g * G:(g + 1) * G, :, :])
                for t in range(G):
                    ht = g * G + t
                    ps = ps_mm.tile([128, TPC], F32, tag="mm")
                    for j in range(NE):
                        nc.tensor.matmul(ps[:], w1_t[:, t, j, :], y16_t[:, j, :],
                                         start=(j == 0), stop=(j == NE - 1))
                    nc.scalar.activation(h16_t[:, ht, :], ps[:], AF.Relu)

            # ---- z = (h W2) * r2 + y ----
            for g in range(4):
                w2_t = w2pool.tile([128, NE // 4, NH, 128], BF16, tag="w2")
                nc.sync.dma_start(w2_t[:], w2_d[:, g * 2:(g + 1) * 2, :, :])
                for t in range(NE // 4):
                    i = g * 2 + t
                    ps = ps_acc.tile([128, TPC], F32, tag="acc")
                    for ht in range(NH):
                        nc.tensor.matmul(ps[:], w2_t[:, t, ht, :], h16_t[:, ht, :],
                                         start=(ht == 0), stop=(ht == NH - 1))
                    zt = spool.tile([128, TPC], F32, tag="zt")
                    nc.vector.tensor_mul(zt[:], ps[:], r2b[:])
                    outt = spool.tile([128, TPC], F32, tag="outt")
                    nc.vector.tensor_add(outt[:], zt[:], y_t[:, i, :])
                    nc.sync.dma_start(out_d[i * 128:(i + 1) * 128, :], outt[:])
    nc.compile()
    return nc


def prep_b_inputs(inputs, oT_all):
    x = np.asarray(inputs["x"], np.float32)
    ln2 = np.asarray(inputs["ln2_w"], np.float32)
    BF16NP = ml_dtypes.bfloat16

    def ktile16(w, M):
        # [K, M] -> [128, M//128, K//128, 128] bf16
        K = w.shape[0]
        return np.ascontiguousarray(
            w.reshape(K // 128, 128, M // 128, 128)
            .transpose(1, 2, 0, 3)).astype(BF16NP)

    Wo = ktile16(np.asarray(inputs["Wo"], np.float32), E)
    W1 = ktile16(ln2[:, None] * np.asarray(inputs["W1"], np.float32), HID)
    W2 = ktile16(np.asarray(inputs["W2"], np.float32), E)
    xT = x.reshape(B * S, E).T  # [E, B*S]
    in_maps = []
    for c in range(NCORE):
        ts = slice(c * TPC, (c + 1) * TPC)
        oT16 = np.ascontiguousarray(
            oT_all[:, ts].reshape(8, 128, TPC).transpose(1, 0, 2)).astype(BF16NP)
        xTs = np.ascontiguousarray(
            xT[:, ts].reshape(8, 128, TPC).transpose(1, 0, 2)).astype(np.float32)
        in_maps.append({
            "oT16": oT16,
            "xTs": xTs,
            "onesb": np.ones((128, 1), BF16NP),
            "onesr": np.ones((1, 128), np.float32),
            "wo16": Wo, "w116": W1, "w216": W2,
        })
    return in_maps


_CACHE = {}


def _get_compiled():
    if "a" not in _CACHE:
        _CACHE["a"] = build_launch_a()
    if "b" not in _CACHE:
        _CACHE["b"] = build_launch_b()
    return _CACHE["a"], _CACHE["b"]


def kernel(**inputs):
    from concourse import bass_utils
    inputs = {k: np.asarray(v) for k, v in inputs.items()}
    nca, ncb = _get_compiled()
    in_maps_a = prep_a_inputs(inputs)
    res_a = bass_utils.run_bass_kernel_spmd(
        nca, in_maps_a, core_ids=list(range(NCORE)))
    oT_all = np.concatenate([res_a.results[c]["oT"] for c in range(NCORE)],
                            axis=0)  # [E, B*S], rows = h*64+d
    in_maps_b = prep_b_inputs(inputs, oT_all)
    res_b = bass_utils.run_bass_kernel_spmd(
        ncb, in_maps_b, core_ids=list(range(NCORE)))
    outT = np.concatenate([res_b.results[c]["outT"] for c in range(NCORE)],
                          axis=1)  # [E, B*S]
    return np.ascontiguousarray(outT.T).reshape(B, S, E).astype(np.float32)



# revision 10
# speedup vs baseline: 1.5467x; 1.3457x over previous
# Trainium2 Bass kernels for nn_DecoderLayer (T5-style decoder layer).
# Two SPMD launches over 8 cores:
#   A: head-parallel attention (2 heads/core) -> per-head attn outputs O^T
#   B: token-parallel output-proj + FFN (512 tokens/core)
# Activations kept feature-major (features on partitions).
import sys
sys.path.insert(0, '/opt/trn_rl_repo')
import math
import numpy as np
import bass_rust
import concourse.bass as bass
import concourse.mybir as mybir
import concourse.tile as tile
from concourse import bacc

F32 = mybir.dt.float32
F32R = mybir.dt.float32r
F8 = mybir.dt.float8e4
BF16 = mybir.dt.bfloat16
AF = mybir.ActivationFunctionType
DRMODE = mybir.MatmulPerfMode.DoubleRow
import ml_dtypes
FP8NP = ml_dtypes.float8_e4m3

E = 1024
H = 16
D = 64
HID = 4096
B = 2
S = 2048
NB = 32
MAXD = 128
LN_EPS = 1e-6
KQ_EPS = 1e-6
NCORE = 8
HPC = H // NCORE          # heads per core (2)
TPC = B * S // NCORE      # tokens per core (512)
NQC = S // 512            # q-chunks per batch (4)
BOFF = 1024               # bias vec offset: index j = d + BOFF
MASKVAL = -20.0


def rel_bucket(d):
    d = np.asarray(d)
    max_exact = NB // 2
    safe = np.maximum(d, 1).astype(np.float64)
    large = max_exact + (
        np.log(safe / max_exact) / math.log(MAXD / max_exact) * (NB - max_exact)
    ).astype(np.int32)
    large = np.minimum(large, NB - 1)
    return np.where(d < max_exact, d, large)


def build_launch_a():
    """Head-parallel attention, fp8 DoubleRow everywhere.

    Per core: 2 heads x 2 batches over S=2048.
    - q/k proj: weight-hilo DR pairs (W_hi, W_lo*64) x (x8, x8/64)
    - v proj: token-major, lhsT = x pairs, rhs = Wv pairs -> [tok, d]
    - scores: DR-32 over d-half pairs (q/k repacked [32, 2, S] via DMA)
    - rel-bias: ident-DR accumulation into score psum (fp8 bias tiles)
    - exp: wide [128, 2, 512] activation psum->fp8 p-pairs
    - PV: DR over kt pairs with ones-row for denominators
    - l2-norm: gpsimd partition_all_reduce + Act sqrt + DVE recip
    - r1 (pre-attn RMS scale) comes precomputed from host (input prep)
    """
    from concourse import bass_isa
    RADD = bass_isa.ReduceOp.add
    MUL = mybir.AluOpType.mult

    nc = bacc.Bacc("TRN2", target_bir_lowering=False, debug=False)
    BS = B * S
    x8d_d = nc.dram_tensor("x8d", [128, 8, 2, BS], F8, kind="ExternalInput").ap()
    wq8_d = nc.dram_tensor("wq8", [128, 8, 2, 128], F8, kind="ExternalInput").ap()
    wk8_d = nc.dram_tensor("wk8", [128, 8, 2, 128], F8, kind="ExternalInput").ap()
    wv8_d = nc.dram_tensor("wv8", [128, 8, 2, 2, 64], F8,
                           kind="ExternalInput").ap()
    bias8_d = nc.dram_tensor("bias8", [128, 13, 512], F8,
                             kind="ExternalInput").ap()
    identp_d = nc.dram_tensor("identp", [128, 2, 128], F8,
                              kind="ExternalInput").ap()
    r1t_d = nc.dram_tensor("r1t", [128, 32], F32, kind="ExternalInput").ap()
    fconst_d = nc.dram_tensor("fconst", [128, HPC], F32,
                              kind="ExternalInput").ap()
    hsum_d = nc.dram_tensor("hsum", [128, HPC], F32R, kind="ExternalInput").ap()
    hsel_d = nc.dram_tensor("hsel", [HPC, 128], F32R, kind="ExternalInput").ap()
    scale2_d = nc.dram_tensor("scale2", [HPC, 1], F32, kind="ExternalInput").ap()
    ones64_d = nc.dram_tensor("ones64", [1, 64], F32R, kind="ExternalInput").ap()
    ones2_d = nc.dram_tensor("ones2", [128, 2, 16], F8, kind="ExternalInput").ap()
    oT_d = nc.dram_tensor("oT", [HPC * D, BS], BF16, kind="ExternalOutput").ap()

    with tile.TileContext(nc) as tc:
        with nc.allow_low_precision(reason="fp8 kernel"), \
             tc.tile_pool(name="const", bufs=1) as cpool, \
             tc.tile_pool(name="xt", bufs=1) as xpool, \
             tc.tile_pool(name="qk", bufs=2) as qkpool, \
             tc.tile_pool(name="qkp", bufs=2) as qkppool, \
             tc.tile_pool(name="vp", bufs=2) as vpool, \
             tc.tile_pool(name="l2", bufs=2) as l2pool, \
             tc.tile_pool(name="pt", bufs=3) as ppool, \
             tc.tile_pool(name="onorm", bufs=3) as opool, \
             tc.tile_pool(name="small", bufs=4) as spool, \
             tc.tile_pool(name="ps_l2", bufs=1, space="PSUM") as ps_l2, \
             tc.tile_pool(name="ps_sc", bufs=2, space="PSUM") as ps_sc, \
             tc.tile_pool(name="ps_o", bufs=1, space="PSUM") as ps_o:

            # ---- constants ----
            wq8 = cpool.tile([128, 8, 2, 128], F8)
            nc.sync.dma_start(wq8[:], wq8_d[:])
            wk8 = cpool.tile([128, 8, 2, 128], F8)
            nc.sync.dma_start(wk8[:], wk8_d[:])
            wv8 = cpool.tile([128, 8, 2, 2, 64], F8)
            nc.sync.dma_start(wv8[:], wv8_d[:])
            bias8 = cpool.tile([128, 13, 512], F8)
            nc.sync.dma_start(bias8[:], bias8_d[:])
            identp = cpool.tile([128, 2, 128], F8)
            nc.sync.dma_start(identp[:], identp_d[:])
            r1t = cpool.tile([128, 32], F32)
            nc.sync.dma_start(r1t[:], r1t_d[:])
            fconst_t = cpool.tile([128, HPC], F32)
            nc.sync.dma_start(fconst_t[:], fconst_d[:])
            hsum = cpool.tile([128, HPC], F32R)
            nc.sync.dma_start(hsum[:], hsum_d[:])
            hsel = cpool.tile([HPC, 128], F32R)
            nc.sync.dma_start(hsel[:], hsel_d[:])
            scale2 = cpool.tile([HPC, 1], F32)
            nc.sync.dma_start(scale2[:], scale2_d[:])
            ones64 = cpool.tile([1, 64], F32R)
            nc.sync.dma_start(ones64[:], ones64_d[:])
            ones2 = cpool.tile([128, 2, 16], F8)
            nc.sync.dma_start(ones2[:], ones2_d[:])
            epskq = cpool.tile([128, 1], F32)
            nc.vector.memset(epskq[:], KQ_EPS)

            # x: one big resident tile, DMA'd in token chunks for pipelining
            x8d = xpool.tile([128, 8, 2, BS], F8, tag="x")
            for b in range(B):
                for ch in range(4):
                    gsl = slice(b * S + ch * 512, b * S + (ch + 1) * 512)
                    nc.sync.dma_start(x8d[:, :, :, gsl], x8d_d[:, :, :, gsl])

            state = {}

            def phase1(b):
                """Generator: yields after each independently-schedulable piece."""
                q8 = qkpool.tile([128, S], F8, tag="q8", name=f"q8_{b}")
                k8 = qkpool.tile([128, S], F8, tag="k8", name=f"k8_{b}")
                vpairs = [[vpool.tile([128, 2, 80], F8, tag=f"v{h}_{p}",
                                      name=f"vp{b}_{h}_{p}")
                           for p in range(8)] for h in range(HPC)]
                for h in range(HPC):
                    for p in range(8):
                        nc.gpsimd.memset(vpairs[h][p][:, :, 64:65], 1.0)

                def l2unit(w_t, dst, isq, ch):
                    csl = slice(ch * 512, (ch + 1) * 512)
                    gsl = slice(b * S + ch * 512, b * S + (ch + 1) * 512)
                    raw = ps_l2.tile([128, 512], F32, tag="raw", name="raw")
                    for i in range(8):
                        nc.tensor.matmul(raw[:], w_t[:, i, :, :],
                                         x8d[:, i, :, gsl],
                                         start=(i == 0), stop=(i == 7),
                                         perf_mode=DRMODE)
                    rawS = l2pool.tile([128, 512], F32, tag="rawS",
                                       name="rawS")
                    nc.vector.tensor_copy(rawS[:], raw[:])
                    sq = l2pool.tile([128, 512], F32R, tag="sq", name="sq")
                    nc.gpsimd.tensor_mul(sq[:], rawS[:], rawS[:])
                    ss_ps = ps_l2.tile([2, 512], F32, tag="ss", name="ss_ps")
                    nc.tensor.matmul(ss_ps[:], hsum[:], sq[:],
                                     start=True, stop=True)
                    rsq = l2pool.tile([2, 512], F32, tag="rsq", name="rsq")
                    nc.scalar.activation(rsq[:], ss_ps[:], AF.Sqrt,
                                         bias=epskq[0:2, :])
                    rn = l2pool.tile([2, 512], F32R, tag="rn", name="rn")
                    nc.vector.reciprocal(rn[:], rsq[:])
                    if isq:
                        nc.vector.tensor_scalar_mul(rn[:], rn[:], scale2[:])
                    rnb_ps = ps_l2.tile([128, 512], F32, tag="rnb",
                                        name="rnb_ps")
                    nc.tensor.matmul(rnb_ps[:], hsel[:], rn[:],
                                     start=True, stop=True)
                    nc.vector.tensor_mul(dst[:, csl], rawS[:], rnb_ps[:])

                def vunit(ch):
                    vps = ps_l2.tile([128, 512], F32, tag="raw", name="vps")
                    for w in range(4):
                        tt = ch * 4 + w
                        t0 = b * S + tt * 128
                        for h in range(HPC):
                            wsl = slice((w * 2 + h) * 64, (w * 2 + h + 1) * 64)
                            for i in range(8):
                                nc.tensor.matmul(
                                    vps[:, wsl], x8d[:, i, :, t0:t0 + 128],
                                    wv8[:, i, :, h, :],
                                    start=(i == 0), stop=(i == 7),
                                    perf_mode=DRMODE)
                            nc.vector.tensor_scalar_mul(
                                vpairs[h][tt // 2][:, tt % 2, 0:64],
                                vps[:, wsl],
                                r1t[:, b * 16 + tt:b * 16 + tt + 1])

                for ch in range(4):
                    l2unit(wq8, q8, True, ch)
                    yield
                    vunit(ch)
                    yield
                    l2unit(wk8, k8, False, ch)
                    yield

                qkp = {}
                for h in range(HPC):
                    for (nm, src) in (("q", q8), ("k", k8)):
                        t = qkppool.tile([32, 2, S], F8, tag=f"{nm}p{h}",
                                         name=f"{nm}p{b}{h}")
                        nc.sync.dma_start(t[:], src[h * 64:(h + 1) * 64, :])
                        qkp[(nm, h)] = t
                yield

                mvv_s = {}
                for h in range(HPC):
                    mv = ps_o.tile([65, 512], F32, tag="o", name="mv")
                    for p in range(8):
                        nc.tensor.matmul(mv[:, 0:1], vpairs[h][p][:, :, 0:65],
                                         ones2[:, :, 0:1],
                                         start=(p == 0), stop=(p == 7),
                                         perf_mode=DRMODE)
                    ms = spool.tile([64, 1], F32, tag="mvv", name="ms")
                    nc.scalar.activation(ms[:], mv[0:64, 0:1], AF.Copy,
                                         scale=1.0 / S)
                    mvv_s[h] = ms
                state[b] = (qkp, vpairs, mvv_s)
                yield

            def phase2(b):
                """Generator yielding after each (h, qc) unit; PV software-
                pipelined one pair behind scores/exp."""
                qkp, vpairs, mvv_s = state[b]
                for h in range(HPC):
                    qp, kp = qkp[("q", h)], qkp[("k", h)]
                    for qc in range(NQC):
                        Q0 = qc * 512
                        qsl = slice(Q0, Q0 + 512)
                        nkt = 4 * qc + 4
                        o_ps = ps_o.tile([65, 512], F32, tag="o", name="o_ps")
                        pending = None  # (p_pair, pk)

                        def flush(last):
                            pp, pk = pending
                            nc.tensor.matmul(
                                o_ps[:], vpairs[h][pk][:, :, 0:65], pp[:],
                                start=(pk == 0), stop=last,
                                perf_mode=DRMODE)

                        for pk in range(nkt // 2):
                            s_ps = ps_sc.tile([128, 1024], F32, tag="sc",
                                              name="s_ps")
                            anynear = any((Q0 - 128 * (2 * pk + hf)) < 255
                                          for hf in range(2))
                            for half in range(2):
                                kt = 2 * pk + half
                                K0 = kt * 128
                                near = (Q0 - K0) < 255
                                ssl = s_ps[:, half * 512:(half + 1) * 512]
                                if near or anynear:
                                    di = (1 + kt - 4 * qc) if near else 5
                                    j = h * 6 + di
                                    nc.tensor.matmul(
                                        ssl, identp[:], bias8[:, j:j + 2, :],
                                        start=True, stop=False,
                                        perf_mode=DRMODE)
                                    nc.tensor.matmul(
                                        ssl, kp[:, :, K0:K0 + 128],
                                        qp[:, :, qsl],
                                        start=False, stop=True,
                                        perf_mode=DRMODE)
                                else:
                                    nc.tensor.matmul(
                                        ssl, kp[:, :, K0:K0 + 128],
                                        qp[:, :, qsl],
                                        start=True, stop=True,
                                        perf_mode=DRMODE)
                            p_pair = ppool.tile([128, 2, 512], F8, tag="p",
                                                name="p_pair")
                            sin = s_ps[:].rearrange("p (i m) -> p i m", i=2)
                            if anynear:
                                nc.scalar.activation(p_pair[:], sin, AF.Exp)
                            else:
                                nc.scalar.activation(p_pair[:], sin, AF.Exp,
                                                     bias=fconst_t[:, h:h + 1])
                            if pending is not None:
                                flush(False)
                            pending = (p_pair, pk)
                        flush(True)
                        pending = None
                        srec = spool.tile([1, 512], F32R, tag="srec",
                                          name="srec")
                        nc.vector.reciprocal(srec[:], o_ps[64:65, :])
                        nb = spool.tile([64, 512], F32R, tag="nb", name="nb")
                        nc.gpsimd.partition_broadcast(nb[:], srec[:], 64)
                        o_n = opool.tile([64, 512], BF16, tag="on", name="o_n")
                        nc.vector.tensor_mul(o_n[:], o_ps[0:64, :], nb[:])
                        if qc == 0:
                            nc.scalar.activation(o_n[:, 0:1], mvv_s[h][:],
                                                 AF.Copy)
                        nc.sync.dma_start(
                            oT_d[h * D:(h + 1) * D,
                                 b * S + Q0:b * S + Q0 + 512],
                            o_n[:])
                        yield

            # interleaved emission: phase1(0); then phase2(0) woven with
            # phase1(1); then phase2(1)
            for _ in phase1(0):
                pass
            g1 = phase1(1)
            done1 = False
            for _ in phase2(0):
                for _ in range(2):
                    if not done1:
                        try:
                            next(g1)
                        except StopIteration:
                            done1 = True
            while not done1:
                try:
                    next(g1)
                except StopIteration:
                    done1 = True
            for _ in phase2(1):
                pass

    nc.compile()
    return nc


def prep_a_inputs(inputs):
    x = np.asarray(inputs["x"], np.float32)
    ln1 = np.asarray(inputs["ln1_w"], np.float32)
    Wq = np.asarray(inputs["Wq"], np.float32)
    Wk = np.asarray(inputs["Wk"], np.float32)
    Wv = np.asarray(inputs["Wv"], np.float32)
    rb = np.asarray(inputs["rel_bias"], np.float32)
    scale = np.asarray(inputs["scale"], np.float32)
    BS = B * S

    xT = np.ascontiguousarray(x.reshape(BS, E).T)  # [E, BS]
    x8 = xT.astype(FP8NP)
    x8s = (xT / 64.0).astype(FP8NP)
    x8d = np.empty((128, 8, 2, BS), FP8NP)
    x8d[:, :, 0, :] = x8.reshape(8, 128, BS).transpose(1, 0, 2)
    x8d[:, :, 1, :] = x8s.reshape(8, 128, BS).transpose(1, 0, 2)

    # host-side r1 (pre-attention RMS scale per token)
    r1 = 1.0 / np.sqrt((x.astype(np.float64) ** 2).mean(-1) + LN_EPS)
    r1 = r1.reshape(BS).astype(np.float32)   # [BS]
    r1t = np.ascontiguousarray(
        r1.reshape(2, 16, 128).transpose(2, 0, 1).reshape(128, 32))

    # bias vector per head (scale-folded) and tiles
    d = np.arange(2048) - BOFF
    bucket = rel_bucket(np.maximum(d, 1))
    biasv_all = np.where(
        (d < 1)[None, :], np.float32(MASKVAL),
        scale[:, None] * rb[bucket, :].T.astype(np.float32)).astype(np.float32)
    di_ = np.arange(5)[:, None, None]
    p_ = np.arange(128)[None, :, None]
    f_ = np.arange(512)[None, None, :]
    idx = BOFF + 128 - 128 * di_ + f_ - p_
    biast_all = biasv_all[:, idx]  # [H, 5, 128, 512]
    fconst_all = (scale * rb[NB - 1, :]).astype(np.float32)

    identp = np.zeros((128, 2, 128), np.float32)
    identp[:, 0, :] = np.eye(128, dtype=np.float32)
    hsum_np = np.zeros((128, HPC), np.float32)
    hsel_np = np.zeros((HPC, 128), np.float32)
    for h in range(HPC):
        hsum_np[h * 64:(h + 1) * 64, h] = 1.0
        hsel_np[h, h * 64:(h + 1) * 64] = 1.0

    def hilo(w):
        hi = w.astype(FP8NP)
        lo = ((w - hi.astype(np.float32)) * 64.0).astype(FP8NP)
        return hi, lo

    in_maps = []
    for c in range(NCORE):
        gh = slice(c * HPC, (c + 1) * HPC)          # global heads
        cs = slice(c * HPC * D, (c + 1) * HPC * D)  # proj out cols
        wq_hi, wq_lo = hilo(ln1[:, None] * Wq[:, cs])
        wk_hi, wk_lo = hilo(ln1[:, None] * Wk[:, cs])
        wv_hi, wv_lo = hilo(ln1[:, None] * Wv[:, cs])

        def pack_qk(hi, lo):
            t = np.empty((128, 8, 2, 128), FP8NP)
            t[:, :, 0, :] = hi.reshape(8, 128, 128).transpose(1, 0, 2)
            t[:, :, 1, :] = lo.reshape(8, 128, 128).transpose(1, 0, 2)
            return t

        def pack_v(hi, lo):
            t = np.empty((128, 8, 2, 2, 64), FP8NP)
            t[:, :, 0, :, :] = hi.reshape(8, 128, 2, 64).transpose(1, 0, 2, 3)
            t[:, :, 1, :, :] = lo.reshape(8, 128, 2, 64).transpose(1, 0, 2, 3)
            return t

        bias8 = np.zeros((128, 13, 512), FP8NP)
        for h in range(HPC):
            g = c * HPC + h
            bias8[:, h * 6:h * 6 + 5, :] = biast_all[g].transpose(
                1, 0, 2).astype(FP8NP)
            bias8[:, h * 6 + 5, :] = np.float32(fconst_all[g]).astype(FP8NP)

        in_maps.append({
            "x8d": x8d,
            "wq8": pack_qk(wq_hi, wq_lo),
            "wk8": pack_qk(wk_hi, wk_lo),
            "wv8": pack_v(wv_hi, wv_lo),
            "bias8": bias8,
            "identp": identp.astype(FP8NP),
            "r1t": r1t,
            "fconst": np.ascontiguousarray(
                np.broadcast_to(fconst_all[gh], (128, HPC))),
            "hsum": hsum_np,
            "hsel": hsel_np,
            "scale2": np.ascontiguousarray(scale[gh, None]),
            "ones64": np.ones((1, 64), np.float32),
            "ones2": np.ones((128, 2, 16), FP8NP),
        })
    return in_maps


def build_launch_b():
    nc = bacc.Bacc("TRN2", target_bir_lowering=False, debug=False)
    NE = E // 128    # 8 e-tiles
    NH = HID // 128  # 32 h-tiles
    oT_d = nc.dram_tensor("oT16", [128, NE, TPC], BF16, kind="ExternalInput").ap()
    xT_d = nc.dram_tensor("xTs", [128, NE, TPC], F32, kind="ExternalInput").ap()
    wo_d = nc.dram_tensor("wo16", [128, NE, NE, 128], BF16,
                          kind="ExternalInput").ap()
    w1_d = nc.dram_tensor("w116", [128, NH, NE, 128], BF16,
                          kind="ExternalInput").ap()
    w2_d = nc.dram_tensor("w216", [128, NE, NH, 128], BF16,
                          kind="ExternalInput").ap()
    ones_d = nc.dram_tensor("onesb", [128, 1], BF16, kind="ExternalInput").ap()
    onesr_d = nc.dram_tensor("onesr", [1, 128], F32R, kind="ExternalInput").ap()
    out_d = nc.dram_tensor("outT", [E, TPC], F32, kind="ExternalOutput").ap()

    with tile.TileContext(nc) as tc:
        with nc.allow_low_precision(reason="bf16 kernel"), \
             tc.tile_pool(name="const", bufs=1) as cpool, \
             tc.tile_pool(name="io", bufs=1) as iopool, \
             tc.tile_pool(name="y", bufs=1) as ypool, \
             tc.tile_pool(name="h", bufs=1) as hpool, \
             tc.tile_pool(name="w1s", bufs=2) as w1pool, \
             tc.tile_pool(name="w2s", bufs=2) as w2pool, \
             tc.tile_pool(name="small", bufs=2) as spool, \
             tc.tile_pool(name="ps_mm", bufs=4, space="PSUM") as ps_mm, \
             tc.tile_pool(name="ps_acc", bufs=2, space="PSUM") as ps_acc, \
             tc.tile_pool(name="ps_r", bufs=1, space="PSUM") as ps_r:

            ones16 = cpool.tile([128, 1], BF16)
            nc.sync.dma_start(ones16[:], ones_d[:])
            o1x128 = cpool.tile([1, 128], F32R)
            nc.sync.dma_start(o1x128[:], onesr_d[:])
            epsln_t = cpool.tile([128, 1], F32)
            nc.vector.memset(epsln_t[:], LN_EPS)

            wo_t = cpool.tile([128, NE, NE, 128], BF16)
            nc.sync.dma_start(wo_t[:], wo_d[:])
            oT16 = iopool.tile([128, NE, TPC], BF16, tag="oT16")
            nc.sync.dma_start(oT16[:], oT_d[:])
            xT8 = iopool.tile([128, NE, TPC], F32, tag="xTs")
            nc.sync.dma_start(xT8[:], xT_d[:])

            # ---- y = Wo^T @ O + x ; y16 bf16 copy; sq16 squares ----
            y_t = ypool.tile([128, NE, TPC], F32, tag="y")
            y16_t = ypool.tile([128, NE, TPC], BF16, tag="y16")
            sq16_t = ypool.tile([128, NE, TPC], BF16, tag="sq16")
            for i in range(NE):
                ps = ps_mm.tile([128, TPC], F32, tag="mm")
                for j in range(NE):
                    nc.tensor.matmul(ps[:], wo_t[:, i, j, :], oT16[:, j, :],
                                     start=(j == 0), stop=(j == NE - 1))
                nc.vector.tensor_add(y_t[:, i, :], ps[:], xT8[:, i, :])
                nc.gpsimd.tensor_copy(y16_t[:, i, :], y_t[:, i, :])
                nc.gpsimd.tensor_mul(sq16_t[:, i, :], y16_t[:, i, :],
                                     y16_t[:, i, :])

            # ---- r2 = 1/sqrt(mean(y^2)+eps), broadcast to [128, TPC] ----
            ssy_ps = ps_r.tile([128, TPC], F32, tag="r")
            for i in range(NE):
                nc.tensor.matmul(ssy_ps[0:1, :], ones16[:], sq16_t[:, i, :],
                                 start=(i == 0), stop=(i == NE - 1))
            r2sq = spool.tile([1, TPC], F32, tag="r2sq")
            nc.scalar.activation(r2sq[:], ssy_ps[0:1, :], AF.Sqrt,
                                 bias=epsln_t[0:1, :], scale=1.0 / E)
            r2 = spool.tile([1, TPC], F32R, tag="r2")
            nc.vector.reciprocal(r2[:], r2sq[:])
            r2b_ps = ps_r.tile([128, TPC], F32, tag="r")
            nc.tensor.matmul(r2b_ps[:], o1x128[:], r2[:], start=True, stop=True)
            r2b = cpool.tile([128, TPC], F32)
            nc.scalar.activation(r2b[:], r2b_ps[:], AF.Copy)

            # ---- h = relu(W1'^T y) in bf16 (weights streamed in 4 groups) ----
            h16_t = hpool.tile([128, NH, TPC], BF16, tag="h16")
            G = NH // 4
            for g in range(4):
                w1_t = w1pool.tile([128, G, NE, 128], BF16, tag="w1")
                nc.sync.dma_start(w1_t[:], w1_d[:, g * G:(g + 1) * G, :, :])
                for t in range(G):
                    ht = g * G + t
                    ps = ps_mm.tile([128, TPC], F32, tag="mm")
                    for j in range(NE):
                        nc.tensor.matmul(ps[:], w1_t[:, t, j, :], y16_t[:, j, :],
                                         start=(j == 0), stop=(j == NE - 1))
                    nc.scalar.activation(h16_t[:, ht, :], ps[:], AF.Relu)

            # ---- z = (h W2) * r2 + y ----
            for g in range(4):
                w2_t = w2pool.tile([128, NE // 4, NH, 128], BF16, tag="w2")
                nc.sync.dma_start(w2_t[:], w2_d[:, g * 2:(g + 1) * 2, :, :])
                for t in range(NE // 4):
                    i = g * 2 + t
                    ps = ps_acc.tile([128, TPC], F32, tag="acc")
                    for ht in range(NH):
                        nc.tensor.matmul(ps[:], w2_t[:, t, ht, :], h16_t[:, ht, :],
                                         start=(ht == 0), stop=(ht == NH - 1))
                    zt = spool.tile([128, TPC], F32, tag="zt")
                    nc.vector.tensor_mul(zt[:], ps[:], r2b[:])
                    outt = spool.tile([128, TPC], F32, tag="outt")
                    nc.vector.tensor_add(outt[:], zt[:], y_t[:, i, :])
                    nc.sync.dma_start(out_d[i * 128:(i + 1) * 128, :], outt[:])
    nc.compile()
    return nc


def prep_b_inputs(inputs, oT_all):
    x = np.asarray(inputs["x"], np.float32)
    ln2 = np.asarray(inputs["ln2_w"], np.float32)
    BF16NP = ml_dtypes.bfloat16

    def ktile16(w, M):
        # [K, M] -> [128, M//128, K//128, 128] bf16
        K = w.shape[0]
        return np.ascontiguousarray(
            w.reshape(K // 128, 128, M // 128, 128)
            .transpose(1, 2, 0, 3)).astype(BF16NP)

    Wo = ktile16(np.asarray(inputs["Wo"], np.float32), E)
    W1 = ktile16(ln2[:, None] * np.asarray(inputs["W1"], np.float32), HID)
    W2 = ktile16(np.asarray(inputs["W2"], np.float32), E)
    xT = x.reshape(B * S, E).T  # [E, B*S]
    in_maps = []
    for c in range(NCORE):
        ts = slice(c * TPC, (c + 1) * TPC)
        oT16 = np.ascontiguousarray(
            oT_all[:, ts].reshape(8, 128, TPC).transpose(1, 0, 2)).astype(BF16NP)
        xTs = np.ascontiguousarray(
            xT[:, ts].reshape(8, 128, TPC).transpose(1, 0, 2)).astype(np.float32)
        in_maps.append({
            "oT16": oT16,
            "xTs": xTs,
            "onesb": np.ones((128, 1), BF16NP),
            "onesr": np.ones((1, 128), np.float32),
            "wo16": Wo, "w116": W1, "w216": W2,
        })
    return in_maps


_CACHE = {}


def _get_compiled():
    if "a" not in _CACHE:
        _CACHE["a"] = build_launch_a()
    if "b" not in _CACHE:
        _CACHE["b"] = build_launch_b()
    return _CACHE["a"], _CACHE["b"]


def kernel(**inputs):
    from concourse import bass_utils
    inputs = {k: np.asarray(v) for k, v in inputs.items()}
    nca, ncb = _get_compiled()
    in_maps_a = prep_a_inputs(inputs)
    res_a = bass_utils.run_bass_kernel_spmd(
        nca, in_maps_a, core_ids=list(range(NCORE)))
    oT_all = np.concatenate([res_a.results[c]["oT"] for c in range(NCORE)],
                            axis=0)  # [E, B*S], rows = h*64+d
    in_maps_b = prep_b_inputs(inputs, oT_all)
    res_b = bass_utils.run_bass_kernel_spmd(
        ncb, in_maps_b, core_ids=list(range(NCORE)))
    outT = np.concatenate([res_b.results[c]["outT"] for c in range(NCORE)],
                          axis=1)  # [E, B*S]
    return np.ascontiguousarray(outT.T).reshape(B, S, E).astype(np.float32)

